# revision 1
# baseline (speedup 1.0000x reference)
"""Trainium2 Bass kernel for nn_EstimatorQNN.

Math reduction: the reference applies a batch-independent 2x2 unitary U
(built from the 4 weights) to |psi> = [cos(th/2), sin(th/2)] with
th = x0 + x1, then returns |amp0|^2 - |amp1|^2.  By unitarity this
collapses to

    out = A*cos(th) + D*sin(th) = R*sin(th + phi)

with A = 2|U00|^2 - 1, D = 2*Re(U00*conj(U01)), R = hypot(A, D),
phi = atan2(A, D).  A/D/R/phi are 4 scalars computed on host from the
weights; the device does the memory-bound elementwise part.

Device chain per element (HW Sin table is only valid on [-pi, pi], so
range-reduce with the fp32 magic-number round trick):
    th' = (x_even + phi) + x_odd              scalar_tensor_tensor
    m   = th'*(1/2pi) + MAGIC                 tensor_scalar
    k2  = (m - MAGIC)*2pi                     tensor_scalar  (= 2pi*round(th'/2pi))
    psi = th' - k2                            tensor_tensor  (in [-pi, pi])
    s   = Sin(psi)                            activation
    y   = s * R                               tensor_scalar

Sharding: pure data parallel over the batch across 8 NeuronCores.
"""

import math

import numpy as np

B_FULL = 8388608
N_CORES = 8
B_SHARD = B_FULL // N_CORES  # 1048576

TILE_F = 2048                      # input tile free dim (elements)
TILE_H = TILE_F // 2               # output tile free dim
N_TILES = (B_SHARD * 2) // (128 * TILE_F)

MAGIC = 12582912.0                 # 1.5 * 2**23: fp32 round-to-nearest-int
TWO_PI = 6.283185307179586
INV_2PI = 1.0 / TWO_PI

# set by kernel() on each call; test.py reads it for profiling info
LAST_RESULT = None


def _host_constants(weights: np.ndarray):
    w = np.asarray(weights, dtype=np.float64)

    def rx(t):
        c, s = np.cos(t / 2), np.sin(t / 2)
        return np.array([[c, -1j * s], [-1j * s, c]], dtype=np.complex128)

    def rz(t):
        return np.array(
            [[np.exp(-1j * t / 2), 0], [0, np.exp(1j * t / 2)]], dtype=np.complex128
        )

    U = np.eye(2, dtype=np.complex128)
    for i in range(len(w) // 2):
        U = rz(w[2 * i + 1]) @ rx(w[2 * i]) @ U
    A = 2.0 * abs(U[0, 0]) ** 2 - 1.0
    D = 2.0 * (U[0, 0] * np.conj(U[0, 1])).real
    R = math.hypot(A, D)
    phi = math.atan2(A, D)
    return float(R), float(phi)


def _build_nc(R: float, phi: float):
    import concourse.bacc as bacc
    import concourse.tile as tile
    from concourse import mybir

    add = mybir.AluOpType.add
    sub = mybir.AluOpType.subtract
    mult = mybir.AluOpType.mult
    f32 = mybir.dt.float32

    nc = bacc.Bacc(
        "TRN2",
        target_bir_lowering=False,
        debug=False,
        enable_asserts=False,
        num_devices=N_CORES,
    )
    x = nc.dram_tensor("x", [B_SHARD, 2], f32, kind="ExternalInput").ap()
    y = nc.dram_tensor("y", [B_SHARD, 1], f32, kind="ExternalOutput").ap()

    # [n, 128, TILE_F] views; each partition line is TILE_F contiguous floats
    xv = x.rearrange("(n p q) t -> n p (q t)", p=128, q=TILE_F // 2)
    yv = y.rearrange("(n p q) o -> n p (q o)", p=128, q=TILE_H)

    with tile.TileContext(nc) as tc:
        with (
            tc.tile_pool(name="inp", bufs=3) as in_pool,
            tc.tile_pool(name="tmp", bufs=2) as tmp_pool,
            tc.tile_pool(name="outp", bufs=3) as out_pool,
        ):
            for i in range(N_TILES):
                t = in_pool.tile([128, TILE_F], f32, tag="t")
                nc.sync.dma_start(t[:], xv[i])

                even = t[:, 0:TILE_F:2]
                odd = t[:, 1:TILE_F:2]

                th = tmp_pool.tile([128, TILE_H], f32, tag="th")
                nc.vector.scalar_tensor_tensor(th[:], even, phi, odd, op0=add, op1=add)

                m = tmp_pool.tile([128, TILE_H], f32, tag="m")
                nc.vector.tensor_scalar(m[:], th[:], INV_2PI, MAGIC, op0=mult, op1=add)

                k2 = tmp_pool.tile([128, TILE_H], f32, tag="k2")
                nc.vector.tensor_scalar(k2[:], m[:], MAGIC, TWO_PI, op0=sub, op1=mult)

                psi = tmp_pool.tile([128, TILE_H], f32, tag="psi")
                nc.vector.tensor_tensor(psi[:], th[:], k2[:], op=sub)

                s = tmp_pool.tile([128, TILE_H], f32, tag="s")
                nc.scalar.activation(
                    s[:], psi[:], mybir.ActivationFunctionType.Sin, bias=0.0, scale=1.0
                )

                o = out_pool.tile([128, TILE_H], f32, tag="o")
                nc.vector.tensor_scalar_mul(o[:], s[:], R)

                nc.sync.dma_start(yv[i], o[:])

    nc.compile()
    return nc


def kernel(inputs: np.ndarray, weights: np.ndarray, _trace: bool = False) -> np.ndarray:
    global LAST_RESULT
    from concourse.bass_utils import run_bass_kernel_spmd

    inputs = np.ascontiguousarray(np.asarray(inputs, dtype=np.float32))
    assert inputs.shape == (B_FULL, 2), inputs.shape

    R, phi = _host_constants(weights)
    nc = _build_nc(R, phi)

    in_maps = [
        {"x": inputs[c * B_SHARD : (c + 1) * B_SHARD]} for c in range(N_CORES)
    ]
    res = run_bass_kernel_spmd(
        nc, in_maps, core_ids=list(range(N_CORES)), trace=_trace
    )
    LAST_RESULT = res
    out = np.concatenate([r["y"] for r in res.results], axis=0)
    return out.astype(np.float32, copy=False)
